# revision 53
# baseline (speedup 1.0000x reference)
"""Multi-head attention with QK-LayerNorm on 8 TRN2 NeuronCores.

Shapes: B=2, T=2048, E=1024, H=16 heads, S=64 head dim.
Sharding: core c handles batch c//4 and the 4 heads [ (c%4)*4 , (c%4)*4+4 ).
Each core computes a partial output (its heads' contribution through Wo);
the host sums the 4 partials per batch and adds bo.

Device-side layout: activations are kept transposed ([feature, t]) so every
matmul contracts over the partition axis without on-device transposes:
  QT/KT   [s(64)*2heads = 128p, T]   (2 tiles per core, 2 heads each)
  V       [t 128p, head, s+1]        (extra ones-column -> softmax row sums)
  scores  S^T [t_k 128p, t_q 512]    (strictly-causal upper blocks skipped)
LayerNorm over s (the partition axis of QT) is done via matmul statistics
(block-diagonal ones lhsT), row math on [16, T] tiles, and a DRAM-roundtrip
partition-broadcast of the per-(head,t) scale/shift rows.
Softmax needs no max-subtraction: LN bounds logits to |q.k| <= ~2.
"""

import json
import math

import numpy as np
import ml_dtypes

import concourse.bass as bass
import concourse.bass2jax as bass2jax
import concourse.bass_utils as bass_utils
import concourse.tile as tile
from concourse import mybir
from concourse.vector_clock import ScopedClock

B, T, E, H, S = 2, 2048, 1024, 16, 64
HPC = 4            # heads per core
EPC = HPC * S      # feature cols per core = 256
LN_EPS = 1e-5
INV4 = float(E) ** -0.25
FP32 = mybir.dt.float32
BF16 = mybir.dt.bfloat16
BF = ml_dtypes.bfloat16

# ---------------------------------------------------------------------------
# Compile hook: this toolchain's walrus accepts at most ONE semaphore wait per
# TPB instruction. Tile attaches several. Split extras into standalone
# EventSemaphore (wait-only) instructions on the same engine.
# ---------------------------------------------------------------------------
_TPB_ENGINES = ("Pool", "Activation", "PE", "DVE", "SP")


def _split_multiwaits(bir_json: bytes) -> bytes:
    d = json.loads(bir_json)
    n_split = 0
    for fn in d.get("functions", []):
        for blk in fn.get("blocks", []):
            insts = blk.get("instructions", [])
            out = []
            for inst in insts:
                si = inst.get("sync_info")
                waits = (si or {}).get("on_wait") or []
                if si and len(waits) > 1 and inst.get("engine") in _TPB_ENGINES:
                    for i, w in enumerate(waits[:-1]):
                        out.append({
                            "debug": inst.get("debug", 0),
                            "engine": inst["engine"],
                            "ins": [],
                            "name": f"{inst['name']}-ws{i}",
                            "opcode": "EventSemaphore",
                            "outs": [],
                            "sync_info": {"on_update": [], "on_wait": [w]},
                        })
                        n_split += 1
                    si["on_wait"] = [waits[-1]]
                out.append(inst)
            blk["instructions"] = out
    return json.dumps(d).encode()


_orig_compile_bir_kernel = bass_utils.compile_bir_kernel


def _patched_compile_bir_kernel(bir_json, tmpdir, neff_name="file.neff"):
    return _orig_compile_bir_kernel(_split_multiwaits(bir_json), tmpdir, neff_name)



bass_utils.compile_bir_kernel = _patched_compile_bir_kernel
bass2jax.compile_bir_kernel = _patched_compile_bir_kernel


def _patched_drain_and_barrier(self, tick_clock, wait_clock):
    # Same as TileContext._drain_and_barrier but the drain's waits are emitted
    # as single-wait instructions (walrus limit).
    gc = tick_clock.global_clock
    ticks = eval(str(gc).replace("VectorClock(", "").rstrip(")"))
    sems = wait_clock.sems.allocated()
    for proc_idx, sem in sems.items():
        t = ticks[proc_idx]
        if t > 0:
            mult = 16 if proc_idx >= 11 else 1
            self.nc.sync.wait_ge(sem, t * mult)
    self.nc.sync.drain()
    self.nc.all_engine_barrier()
    assert self.sems is not None
    popped = self.nc._tile_sem_poison_stack.pop()
    assert popped is self._sem_poison
    self.nc.clear_and_free_semaphores(list(self.sems.allocated().values()))
    self.nc.all_engine_barrier()


tile.TileContext._drain_and_barrier = _patched_drain_and_barrier


# ---------------------------------------------------------------------------
# Device kernel (identical program on all 8 cores)
# ---------------------------------------------------------------------------


def _act_raw(nc, out, in_, func):
    # nc.scalar.activation refuses Reciprocal (accuracy); our tolerance is
    # 2e-2 so the LUT version is fine. Emit InstActivation directly.
    eng = nc.scalar
    inputs = [eng.lower_ap(in_)]
    for arg in (0.0, 1.0, 0.0):  # bias, scale, alpha
        inputs.append(mybir.ImmediateValue(dtype=mybir.dt.float32, value=arg))
    return eng.add_instruction(
        mybir.InstActivation(
            name=nc.get_next_instruction_name(),
            func=func,
            ins=inputs,
            outs=[eng.lower_ap(out)],
        )
    )


def _build_bass():
    nc = bass.Bass()
    xtq_e = nc.dram_tensor("xtq", [128, 8, T], BF16, kind="ExternalInput")
    xtk_e = nc.dram_tensor("xtk", [128, 8, T], BF16, kind="ExternalInput")
    xtv_e = nc.dram_tensor("xtv", [128, 8, T], BF16, kind="ExternalInput")
    wq_e = nc.dram_tensor("wq", [128, 8, EPC], BF16, kind="ExternalInput")
    wk_e = nc.dram_tensor("wk", [128, 8, EPC], BF16, kind="ExternalInput")
    wv_e = nc.dram_tensor("wv", [128, 8, EPC], BF16, kind="ExternalInput")
    wo_e = nc.dram_tensor("wo", [128, 2, E], BF16, kind="ExternalInput")
    masks_e = nc.dram_tensor("masks", [128, 4, 1024], BF16, kind="ExternalInput")
    eye_e = nc.dram_tensor("eye2", [128, 2], BF16, kind="ExternalInput")
    wb_e = nc.dram_tensor("wbcols", [128, 4], FP32, kind="ExternalInput")
    selrep_e = nc.dram_tensor("selrep", [128, 128], BF16, kind="ExternalInput")
    selh_e = nc.dram_tensor("selh", [2, 128], BF16, kind="ExternalInput")
    out_e = nc.dram_tensor("out", [T, E], FP32, kind="ExternalOutput")

    xtq, xtk, xtv = xtq_e.ap(), xtk_e.ap(), xtv_e.ap()
    wq_a, wk_a, wv_a, wo_a = wq_e.ap(), wk_e.ap(), wv_e.ap(), wo_e.ap()

    with tile.TileContext(nc) as tc:
        with tc.tile_pool(name="singles", bufs=1) as singles, \
             tc.tile_pool(name="xstream", bufs=8) as xstream, \
             tc.tile_pool(name="work", bufs=1) as work, \
             tc.tile_pool(name="rows", bufs=1) as rows, \
             tc.tile_pool(name="expp", bufs=8) as expp, \
             tc.tile_pool(name="outp", bufs=3) as outp, \
             tc.tile_pool(name="otsbp", bufs=3) as otsbp, \
             tc.tile_pool(name="rcp", bufs=6) as rcpp, \
             tc.tile_pool(name="rbp", bufs=2) as rbp, \
             tc.tile_pool(name="psu", bufs=2, space="PSUM") as psu, \
             tc.tile_pool(name="psu1", bufs=4, space="PSUM") as psu1:

            # ---- resident constants (issue order = DMA priority) ---------
            wq_sb = singles.tile([128, 8, EPC], BF16)
            wk_sb = singles.tile([128, 8, EPC], BF16)
            eye_sb = singles.tile([128, 2], BF16)
            nc.sync.dma_start(out=eye_sb, in_=eye_e.ap())
            wb_sb = singles.tile([128, 4], FP32)
            nc.sync.dma_start(out=wb_sb, in_=wb_e.ap())
            selrep_sb = singles.tile([128, 128], BF16)
            nc.sync.dma_start(out=selrep_sb, in_=selrep_e.ap())
            selh0_sb = singles.tile([1, 128], BF16)
            nc.sync.dma_start(out=selh0_sb, in_=selh_e.ap()[0:1, :])
            selh1_sb = singles.tile([1, 128], BF16)
            nc.sync.dma_start(out=selh1_sb, in_=selh_e.ap()[1:2, :])
            xtv_sb = singles.tile([128, 8, T], BF16)
            wv_sb = singles.tile([128, 8, EPC], BF16)
            masks_sb = singles.tile([128, 4, 1024], BF16)
            wo_sb = singles.tile([128, 2, E], BF16)

            qt = [singles.tile([128, T], BF16, tag=f"qt{m}", name=f"qt{m}") for m in range(2)]
            kt = [singles.tile([128, T], BF16, tag=f"kt{m}", name=f"kt{m}") for m in range(2)]
            vhat = singles.tile([128, 16, HPC, S + 1], BF16)
            otb = [singles.tile([128, T], BF16, tag=f"otb{m}", name=f"otb{m}") for m in range(2)]
            nc.vector.memset(vhat[:, :, :, S:S + 1], 1.0)

            # ---- Q/K projections + LN statistics (interleaved) -----------
            sums_t = rows.tile([128, T], FP32)
            sumsq_t = rows.tile([128, T], FP32)

            def ln_stats(src_t, m, c):
                sq = work.tile([128, T], BF16, tag="sq")
                nc.vector.tensor_tensor(out=sq, in0=src_t[m], in1=src_t[m],
                                        op=mybir.AluOpType.mult)
                for n in range(4):
                    sl = slice(n * 512, (n + 1) * 512)
                    ps_s = psu1.tile([128, 512], FP32, tag="u1", name="st_s")
                    ps_q = psu1.tile([128, 512], FP32, tag="u1", name="st_q")
                    nc.tensor.matmul(ps_s[0:2, :], lhsT=eye_sb, rhs=src_t[m][:, sl],
                                     start=True, stop=True)
                    nc.tensor.matmul(ps_q[0:2, :], lhsT=eye_sb, rhs=sq[:, sl],
                                     start=True, stop=True)
                    if n % 2 == 0:
                        nc.scalar.activation(out=sums_t[32 * c:32 * c + 2, sl],
                                             in_=ps_s[0:2, :],
                                             func=mybir.ActivationFunctionType.Copy)
                        nc.scalar.activation(out=sumsq_t[32 * c:32 * c + 2, sl],
                                             in_=ps_q[0:2, :],
                                             func=mybir.ActivationFunctionType.Copy)
                    else:
                        nc.vector.tensor_copy(out=sums_t[32 * c:32 * c + 2, sl],
                                              in_=ps_s[0:2, :])
                        nc.vector.tensor_copy(out=sumsq_t[32 * c:32 * c + 2, sl],
                                              in_=ps_q[0:2, :])

            for qk_i, (x_ap, w_sb, dst) in enumerate(((xtq, wq_sb, qt), (xtk, wk_sb, kt))):
                if qk_i == 1:
                    nc.sync.dma_start(out=wk_sb, in_=wk_a)
                xcs = {}
                for m in range(2):
                    if qk_i == 1 and m == 1:
                        nc.sync.dma_start(out=wv_sb, in_=wv_a)
                        nc.sync.dma_start(out=masks_sb, in_=masks_e.ap())
                        nc.sync.dma_start(out=wo_sb, in_=wo_a)
                    pss = [psu.tile([128, 1024], FP32, tag="u", name=f"pss{j}")
                           for j in range(2)]
                    for e8 in range(8):
                        if m == 0:
                            if qk_i == 0:
                                nc.sync.dma_start(out=wq_sb[:, e8, :],
                                                  in_=wq_a[:, e8, :])
                            xc = xstream.tile([128, T], BF16, tag="xchunk",
                                              name=f"xc{e8}")
                            nc.sync.dma_start(out=xc, in_=x_ap[:, e8, :])
                            if qk_i == 1:
                                nc.sync.dma_start(out=xtv_sb[:, e8, :],
                                                  in_=xtv[:, e8, :])
                            xcs[e8] = xc
                        xc = xcs[e8]
                        for n in range(4):
                            nc.tensor.matmul(
                                pss[n // 2][:, (n % 2) * 512:(n % 2) * 512 + 512],
                                lhsT=w_sb[:, e8, m * 128:(m + 1) * 128],
                                rhs=xc[:, n * 512:(n + 1) * 512],
                                start=(e8 == 0), stop=(e8 == 7))
                    for j in range(2):
                        nc.vector.tensor_copy(
                            out=dst[m][:, j * 1024:(j + 1) * 1024], in_=pss[j])
                    ln_stats(dst, m, 2 * qk_i + m)

            # ---- LN row math (overlaps V projection) --------------------
            eps_col = singles.tile([128, 1], FP32)
            nc.vector.memset(eps_col, LN_EPS)
            nc.vector.tensor_scalar_mul(sums_t, sums_t, 1.0 / S)          # mu
            nc.vector.tensor_scalar_mul(sumsq_t, sumsq_t, 1.0 / S)
            tmp = rows.tile([128, T], FP32)
            nc.vector.tensor_tensor(out=tmp, in0=sums_t, in1=sums_t,
                                    op=mybir.AluOpType.mult)
            nc.vector.tensor_tensor(out=sumsq_t, in0=sumsq_t, in1=tmp,
                                    op=mybir.AluOpType.subtract)
            nc.vector.tensor_scalar_max(sumsq_t, sumsq_t, 0.0)
            nc.scalar.activation(out=sumsq_t, in_=sumsq_t,
                                 func=mybir.ActivationFunctionType.Sqrt,
                                 bias=eps_col)
            _act_raw(nc, sumsq_t, sumsq_t,
                     mybir.ActivationFunctionType.Reciprocal)             # rstd
            nc.vector.tensor_tensor(out=tmp, in0=sums_t, in1=sumsq_t,
                                    op=mybir.AluOpType.mult)              # mu*rstd
            c_bfrows = rows.tile([128, T], BF16)
            a_bfrows = rows.tile([128, T], BF16)
            nc.vector.tensor_copy(out=c_bfrows, in_=tmp)
            nc.vector.tensor_copy(out=a_bfrows, in_=sumsq_t)

            # ---- V projection (natural layout + ones column) -------------
            for t16 in range(16):
                psv = psu.tile([128, 1024], FP32, tag="u", name="psv")
                for e8 in range(8):
                    nc.tensor.matmul(
                        psv[:, 0:EPC], lhsT=xtv_sb[:, e8, t16 * 128:(t16 + 1) * 128],
                        rhs=wv_sb[:, e8, :], start=(e8 == 0), stop=(e8 == 7))
                nc.scalar.activation(
                    out=vhat[:, t16, :, 0:S],
                    in_=psv[:, 0:EPC].rearrange("p (h s) -> p h s", h=HPC),
                    func=mybir.ActivationFunctionType.Copy)

            # ---- LN apply via PE row-broadcast --------------------------
            # bp[:, 0:512] = a-row broadcast, bp[:, 512:1024] = c-row; the
            # selector lhsT lives at the same 32-aligned base as the rows.
            def ln_apply(src_t, m, c):
                sel = selrep_sb[32 * c:32 * c + 2, :]
                wcol = wb_sb[:, 0:1] if src_t is qt else wb_sb[:, 2:3]
                bcol = wb_sb[:, 1:2] if src_t is qt else wb_sb[:, 3:4]
                for ch in range(4):
                    sl = slice(ch * 512, (ch + 1) * 512)
                    bpa = psu1.tile([128, 512], FP32, tag="u1", name="bpa")
                    bpc = psu1.tile([128, 512], FP32, tag="u1", name="bpc")
                    nc.tensor.matmul(bpa, lhsT=sel,
                                     rhs=a_bfrows[32 * c:32 * c + 2, sl],
                                     start=True, stop=True,
                                     tile_position=(32 * c, 0))
                    nc.tensor.matmul(bpc, lhsT=sel,
                                     rhs=c_bfrows[32 * c:32 * c + 2, sl],
                                     start=True, stop=True,
                                     tile_position=(32 * c, 0))
                    nc.vector.tensor_tensor(out=src_t[m][:, sl], in0=src_t[m][:, sl],
                                            in1=bpa,
                                            op=mybir.AluOpType.mult)
                    nc.vector.tensor_tensor(out=src_t[m][:, sl], in0=src_t[m][:, sl],
                                            in1=bpc,
                                            op=mybir.AluOpType.subtract)
                nc.vector.tensor_scalar(out=src_t[m], in0=src_t[m],
                                        scalar1=wcol, scalar2=bcol,
                                        op0=mybir.AluOpType.mult,
                                        op1=mybir.AluOpType.add)

            # ---- attention (two head-pair streams interleaved) -----------
            ln_apply(qt, 0, 0)
            ln_apply(kt, 0, 2)
            ln_apply(qt, 1, 1)
            ln_apply(kt, 1, 3)

            def finish_norm(state):
                m_, qb_, otsb_, rcs_ = state
                nb = psu1.tile([128, 512], FP32, tag="u1", name="nb")
                nc.tensor.matmul(nb, lhsT=selh0_sb, rhs=rcs_[0],
                                 start=True, stop=False)
                nc.tensor.matmul(nb, lhsT=selh1_sb, rhs=rcs_[1],
                                 start=False, stop=True)
                rb = rbp.tile([128, 512], FP32, tag="rb")
                _act_raw(nc, rb, nb, mybir.ActivationFunctionType.Reciprocal)
                nc.vector.tensor_tensor(
                    out=otb[m_][:, qb_ * 512:(qb_ + 1) * 512],
                    in0=otsb_, in1=rb, op=mybir.AluOpType.mult)

            def emit_wo(t16):
                pso = psu.tile([128, 1024], FP32, tag="u", name="pso")
                for e2 in range(2):
                    for mm in range(2):
                        nc.tensor.matmul(
                            pso[:, e2 * 512:(e2 + 1) * 512],
                            lhsT=otb[mm][:, t16 * 128:(t16 + 1) * 128],
                            rhs=wo_sb[:, mm, e2 * 512:(e2 + 1) * 512],
                            start=(mm == 0), stop=(mm == 1))
                osb = outp.tile([128, 1024], FP32, tag="osb")
                if t16 % 2 == 0:
                    nc.vector.tensor_copy(out=osb, in_=pso)
                else:
                    nc.scalar.activation(out=osb, in_=pso,
                                         func=mybir.ActivationFunctionType.Copy)
                nc.sync.dma_start(
                    out=out_e.ap()[t16 * 128:(t16 + 1) * 128, :],
                    in_=osb)

            def attn_stream(m):
                for qb in range(4):
                    otps = [psu1.tile([128, 512], FP32, tag="u1",
                                      name=f"otp{m}{h_}") for h_ in range(2)]
                    nkb = 4 * qb + 4
                    exq = []
                    for kb in range(nkb):
                        st = psu.tile([128, 1024], FP32, tag="u", name="st")
                        for h in range(2):
                            pa = slice(64 * h, 64 * h + 64)
                            nc.tensor.matmul(
                                st[:, h * 512:(h + 1) * 512],
                                lhsT=kt[m][pa, kb * 128:(kb + 1) * 128],
                                rhs=qt[m][pa, qb * 512:(qb + 1) * 512],
                                start=True, stop=True)
                        ex = expp.tile([128, 1024], BF16, tag="exp")
                        nc.scalar.activation(
                            out=ex, in_=st,
                            func=mybir.ActivationFunctionType.Exp)
                        d = kb - 4 * qb
                        if d >= 0:  # diagonal block: causal 0/1 mask
                            nc.vector.tensor_tensor(
                                out=ex, in0=ex, in1=masks_sb[:, d, :],
                                op=mybir.AluOpType.mult)
                        exq.append((ex, kb))
                        if len(exq) > 3:
                            exp_, kb_ = exq.pop(0)
                            for h in range(2):
                                nc.tensor.matmul(
                                    otps[h][0:S + 1, :],
                                    lhsT=vhat[:, kb_, 2 * m + h, :],
                                    rhs=exp_[:, h * 512:(h + 1) * 512],
                                    start=(kb_ == 0), stop=False)
                        yield None
                    while exq:
                        exp_, kb_ = exq.pop(0)
                        for h in range(2):
                            nc.tensor.matmul(
                                otps[h][0:S + 1, :],
                                lhsT=vhat[:, kb_, 2 * m + h, :],
                                rhs=exp_[:, h * 512:(h + 1) * 512],
                                start=(kb_ == 0), stop=(kb_ == nkb - 1))
                    # evict O^T + sums rows, then finish normalize on-chip
                    otsb = otsbp.tile([128, 512], FP32, tag="otsb")
                    rcs = []
                    for h in range(2):
                        rc = rcpp.tile([1, 512], BF16, tag="rc", name=f"rc{h}")
                        nc.scalar.activation(out=rc, in_=otps[h][S:S + 1, :],
                                             func=mybir.ActivationFunctionType.Copy)
                        nc.scalar.activation(out=otsb[64 * h:64 * h + 64, :],
                                             in_=otps[h][0:S, :],
                                             func=mybir.ActivationFunctionType.Copy)
                        rcs.append(rc)
                    finish_norm((m, qb, otsb, rcs))
                    yield qb

            g0, g1 = attn_stream(0), attn_stream(1)
            next(g0)
            next(g0)
            done0 = done1 = False
            while not (done0 and done1):
                if not done0:
                    try:
                        next(g0)
                    except StopIteration:
                        done0 = True
                if not done1:
                    try:
                        next(g1)
                    except StopIteration:
                        done1 = True
            for t16 in range(16):
                emit_wo(t16)
    return nc




# revision 55
# speedup vs baseline: 1.0419x; 1.0419x over previous
"""Multi-head attention with QK-LayerNorm on 8 TRN2 NeuronCores.

Shapes: B=2, T=2048, E=1024, H=16 heads, S=64 head dim.
Sharding: core c handles batch c//4 and the 4 heads [ (c%4)*4 , (c%4)*4+4 ).
Each core computes a partial output (its heads' contribution through Wo);
the host sums the 4 partials per batch and adds bo.

Device-side layout: activations are kept transposed ([feature, t]) so every
matmul contracts over the partition axis without on-device transposes:
  QT/KT   [s(64)*2heads = 128p, T]   (2 tiles per core, 2 heads each)
  V       [t 128p, head, s+1]        (extra ones-column -> softmax row sums)
  scores  S^T [t_k 128p, t_q 512]    (strictly-causal upper blocks skipped)
LayerNorm over s (the partition axis of QT) is done via matmul statistics
(block-diagonal ones lhsT), row math on [16, T] tiles, and a DRAM-roundtrip
partition-broadcast of the per-(head,t) scale/shift rows.
Softmax needs no max-subtraction: LN bounds logits to |q.k| <= ~2.
"""

import json
import math

import numpy as np
import ml_dtypes

import concourse.bass as bass
import concourse.bass2jax as bass2jax
import concourse.bass_utils as bass_utils
import concourse.tile as tile
from concourse import mybir
from concourse.vector_clock import ScopedClock

B, T, E, H, S = 2, 2048, 1024, 16, 64
HPC = 4            # heads per core
EPC = HPC * S      # feature cols per core = 256
LN_EPS = 1e-5
INV4 = float(E) ** -0.25
FP32 = mybir.dt.float32
BF16 = mybir.dt.bfloat16
BF = ml_dtypes.bfloat16

# ---------------------------------------------------------------------------
# Compile hook: this toolchain's walrus accepts at most ONE semaphore wait per
# TPB instruction. Tile attaches several. Split extras into standalone
# EventSemaphore (wait-only) instructions on the same engine.
# ---------------------------------------------------------------------------
_TPB_ENGINES = ("Pool", "Activation", "PE", "DVE", "SP")


def _split_multiwaits(bir_json: bytes) -> bytes:
    d = json.loads(bir_json)
    n_split = 0
    for fn in d.get("functions", []):
        for blk in fn.get("blocks", []):
            insts = blk.get("instructions", [])
            out = []
            for inst in insts:
                si = inst.get("sync_info")
                waits = (si or {}).get("on_wait") or []
                if si and len(waits) > 1 and inst.get("engine") in _TPB_ENGINES:
                    for i, w in enumerate(waits[:-1]):
                        out.append({
                            "debug": inst.get("debug", 0),
                            "engine": inst["engine"],
                            "ins": [],
                            "name": f"{inst['name']}-ws{i}",
                            "opcode": "EventSemaphore",
                            "outs": [],
                            "sync_info": {"on_update": [], "on_wait": [w]},
                        })
                        n_split += 1
                    si["on_wait"] = [waits[-1]]
                out.append(inst)
            blk["instructions"] = out
    return json.dumps(d).encode()


_orig_compile_bir_kernel = bass_utils.compile_bir_kernel


def _patched_compile_bir_kernel(bir_json, tmpdir, neff_name="file.neff"):
    return _orig_compile_bir_kernel(_split_multiwaits(bir_json), tmpdir, neff_name)



bass_utils.compile_bir_kernel = _patched_compile_bir_kernel
bass2jax.compile_bir_kernel = _patched_compile_bir_kernel


def _patched_drain_and_barrier(self, tick_clock, wait_clock):
    # Same as TileContext._drain_and_barrier but the drain's waits are emitted
    # as single-wait instructions (walrus limit).
    gc = tick_clock.global_clock
    ticks = eval(str(gc).replace("VectorClock(", "").rstrip(")"))
    sems = wait_clock.sems.allocated()
    for proc_idx, sem in sems.items():
        t = ticks[proc_idx]
        if t > 0:
            mult = 16 if proc_idx >= 11 else 1
            self.nc.sync.wait_ge(sem, t * mult)
    self.nc.sync.drain()
    self.nc.all_engine_barrier()
    assert self.sems is not None
    popped = self.nc._tile_sem_poison_stack.pop()
    assert popped is self._sem_poison
    self.nc.clear_and_free_semaphores(list(self.sems.allocated().values()))
    self.nc.all_engine_barrier()


tile.TileContext._drain_and_barrier = _patched_drain_and_barrier


# ---------------------------------------------------------------------------
# Device kernel (identical program on all 8 cores)
# ---------------------------------------------------------------------------


def _act_raw(nc, out, in_, func):
    # nc.scalar.activation refuses Reciprocal (accuracy); our tolerance is
    # 2e-2 so the LUT version is fine. Emit InstActivation directly.
    eng = nc.scalar
    inputs = [eng.lower_ap(in_)]
    for arg in (0.0, 1.0, 0.0):  # bias, scale, alpha
        inputs.append(mybir.ImmediateValue(dtype=mybir.dt.float32, value=arg))
    return eng.add_instruction(
        mybir.InstActivation(
            name=nc.get_next_instruction_name(),
            func=func,
            ins=inputs,
            outs=[eng.lower_ap(out)],
        )
    )


def _build_bass():
    nc = bass.Bass()
    xtq_e = nc.dram_tensor("xtq", [128, 8, T], BF16, kind="ExternalInput")
    xtk_e = nc.dram_tensor("xtk", [128, 8, T], BF16, kind="ExternalInput")
    xtv_e = nc.dram_tensor("xtv", [128, 8, T], BF16, kind="ExternalInput")
    wq_e = nc.dram_tensor("wq", [128, 8, EPC], BF16, kind="ExternalInput")
    wk_e = nc.dram_tensor("wk", [128, 8, EPC], BF16, kind="ExternalInput")
    wv_e = nc.dram_tensor("wv", [128, 8, EPC], BF16, kind="ExternalInput")
    wo_e = nc.dram_tensor("wo", [128, 2, E], BF16, kind="ExternalInput")
    masks_e = nc.dram_tensor("masks", [128, 4, 1024], BF16, kind="ExternalInput")
    eye_e = nc.dram_tensor("eye2", [128, 2], BF16, kind="ExternalInput")
    wb_e = nc.dram_tensor("wbcols", [128, 4], FP32, kind="ExternalInput")
    selrep_e = nc.dram_tensor("selrep", [128, 128], BF16, kind="ExternalInput")
    selh_e = nc.dram_tensor("selh", [2, 128], BF16, kind="ExternalInput")
    out_e = nc.dram_tensor("out", [T, E], FP32, kind="ExternalOutput")

    xtq, xtk, xtv = xtq_e.ap(), xtk_e.ap(), xtv_e.ap()
    wq_a, wk_a, wv_a, wo_a = wq_e.ap(), wk_e.ap(), wv_e.ap(), wo_e.ap()

    with tile.TileContext(nc) as tc:
        with tc.tile_pool(name="singles", bufs=1) as singles, \
             tc.tile_pool(name="xstream", bufs=8) as xstream, \
             tc.tile_pool(name="work", bufs=1) as work, \
             tc.tile_pool(name="rows", bufs=1) as rows, \
             tc.tile_pool(name="expp", bufs=8) as expp, \
             tc.tile_pool(name="outp", bufs=3) as outp, \
             tc.tile_pool(name="otsbp", bufs=3) as otsbp, \
             tc.tile_pool(name="rcp", bufs=6) as rcpp, \
             tc.tile_pool(name="rbp", bufs=2) as rbp, \
             tc.tile_pool(name="psu", bufs=2, space="PSUM") as psu, \
             tc.tile_pool(name="psu1", bufs=4, space="PSUM") as psu1:

            # ---- resident constants (issue order = DMA priority) ---------
            wq_sb = singles.tile([128, 8, EPC], BF16)
            wk_sb = singles.tile([128, 8, EPC], BF16)
            eye_sb = singles.tile([128, 2], BF16)
            nc.scalar.dma_start(out=eye_sb, in_=eye_e.ap())
            wb_sb = singles.tile([128, 4], FP32)
            nc.scalar.dma_start(out=wb_sb, in_=wb_e.ap())
            selrep_sb = singles.tile([128, 128], BF16)
            nc.scalar.dma_start(out=selrep_sb, in_=selrep_e.ap())
            selh0_sb = singles.tile([1, 128], BF16)
            nc.scalar.dma_start(out=selh0_sb, in_=selh_e.ap()[0:1, :])
            selh1_sb = singles.tile([1, 128], BF16)
            nc.scalar.dma_start(out=selh1_sb, in_=selh_e.ap()[1:2, :])
            xtv_sb = singles.tile([128, 8, T], BF16)
            wv_sb = singles.tile([128, 8, EPC], BF16)
            masks_sb = singles.tile([128, 4, 1024], BF16)
            wo_sb = singles.tile([128, 2, E], BF16)

            qt = [singles.tile([128, T], BF16, tag=f"qt{m}", name=f"qt{m}") for m in range(2)]
            kt = [singles.tile([128, T], BF16, tag=f"kt{m}", name=f"kt{m}") for m in range(2)]
            vhat = singles.tile([128, 16, HPC, S + 1], BF16)
            otb = [singles.tile([128, T], BF16, tag=f"otb{m}", name=f"otb{m}") for m in range(2)]
            nc.vector.memset(vhat[:, :, :, S:S + 1], 1.0)

            # ---- Q/K projections + LN statistics (interleaved) -----------
            sums_t = rows.tile([128, T], FP32)
            sumsq_t = rows.tile([128, T], FP32)

            def ln_stats(src_t, m, c):
                sq = work.tile([128, T], BF16, tag="sq")
                nc.vector.tensor_tensor(out=sq, in0=src_t[m], in1=src_t[m],
                                        op=mybir.AluOpType.mult)
                for n in range(4):
                    sl = slice(n * 512, (n + 1) * 512)
                    ps_s = psu1.tile([128, 512], FP32, tag="u1", name="st_s")
                    ps_q = psu1.tile([128, 512], FP32, tag="u1", name="st_q")
                    nc.tensor.matmul(ps_s[0:2, :], lhsT=eye_sb, rhs=src_t[m][:, sl],
                                     start=True, stop=True)
                    nc.tensor.matmul(ps_q[0:2, :], lhsT=eye_sb, rhs=sq[:, sl],
                                     start=True, stop=True)
                    if n % 2 == 0:
                        nc.scalar.activation(out=sums_t[32 * c:32 * c + 2, sl],
                                             in_=ps_s[0:2, :],
                                             func=mybir.ActivationFunctionType.Copy)
                        nc.scalar.activation(out=sumsq_t[32 * c:32 * c + 2, sl],
                                             in_=ps_q[0:2, :],
                                             func=mybir.ActivationFunctionType.Copy)
                    else:
                        nc.vector.tensor_copy(out=sums_t[32 * c:32 * c + 2, sl],
                                              in_=ps_s[0:2, :])
                        nc.vector.tensor_copy(out=sumsq_t[32 * c:32 * c + 2, sl],
                                              in_=ps_q[0:2, :])

            for qk_i, (x_ap, w_sb, dst) in enumerate(((xtq, wq_sb, qt), (xtk, wk_sb, kt))):
                if qk_i == 1:
                    nc.sync.dma_start(out=wk_sb, in_=wk_a)
                xcs = {}
                for m in range(2):
                    if qk_i == 1 and m == 1:
                        nc.sync.dma_start(out=wv_sb, in_=wv_a)
                        nc.sync.dma_start(out=masks_sb, in_=masks_e.ap())
                        nc.sync.dma_start(out=wo_sb, in_=wo_a)
                    pss = [psu.tile([128, 1024], FP32, tag="u", name=f"pss{j}")
                           for j in range(2)]
                    for e8 in range(8):
                        if m == 0:
                            if qk_i == 0:
                                nc.sync.dma_start(out=wq_sb[:, e8, :],
                                                  in_=wq_a[:, e8, :])
                            xc = xstream.tile([128, T], BF16, tag="xchunk",
                                              name=f"xc{e8}")
                            nc.sync.dma_start(out=xc, in_=x_ap[:, e8, :])
                            if qk_i == 1:
                                nc.sync.dma_start(out=xtv_sb[:, e8, :],
                                                  in_=xtv[:, e8, :])
                            xcs[e8] = xc
                        xc = xcs[e8]
                        for n in range(4):
                            nc.tensor.matmul(
                                pss[n // 2][:, (n % 2) * 512:(n % 2) * 512 + 512],
                                lhsT=w_sb[:, e8, m * 128:(m + 1) * 128],
                                rhs=xc[:, n * 512:(n + 1) * 512],
                                start=(e8 == 0), stop=(e8 == 7))
                    for j in range(2):
                        nc.vector.tensor_copy(
                            out=dst[m][:, j * 1024:(j + 1) * 1024], in_=pss[j])
                    ln_stats(dst, m, 2 * qk_i + m)

            # ---- LN row math (overlaps V projection) --------------------
            eps_col = singles.tile([128, 1], FP32)
            nc.vector.memset(eps_col, LN_EPS)
            nc.vector.tensor_scalar_mul(sums_t, sums_t, 1.0 / S)          # mu
            nc.vector.tensor_scalar_mul(sumsq_t, sumsq_t, 1.0 / S)
            tmp = rows.tile([128, T], FP32)
            nc.vector.tensor_tensor(out=tmp, in0=sums_t, in1=sums_t,
                                    op=mybir.AluOpType.mult)
            nc.vector.tensor_tensor(out=sumsq_t, in0=sumsq_t, in1=tmp,
                                    op=mybir.AluOpType.subtract)
            nc.vector.tensor_scalar_max(sumsq_t, sumsq_t, 0.0)
            nc.scalar.activation(out=sumsq_t, in_=sumsq_t,
                                 func=mybir.ActivationFunctionType.Sqrt,
                                 bias=eps_col)
            _act_raw(nc, sumsq_t, sumsq_t,
                     mybir.ActivationFunctionType.Reciprocal)             # rstd
            nc.vector.tensor_tensor(out=tmp, in0=sums_t, in1=sumsq_t,
                                    op=mybir.AluOpType.mult)              # mu*rstd
            c_bfrows = rows.tile([128, T], BF16)
            a_bfrows = rows.tile([128, T], BF16)
            nc.vector.tensor_copy(out=c_bfrows, in_=tmp)
            nc.vector.tensor_copy(out=a_bfrows, in_=sumsq_t)

            # ---- V projection (natural layout + ones column) -------------
            for t16 in range(16):
                psv = psu.tile([128, 1024], FP32, tag="u", name="psv")
                for e8 in range(8):
                    nc.tensor.matmul(
                        psv[:, 0:EPC], lhsT=xtv_sb[:, e8, t16 * 128:(t16 + 1) * 128],
                        rhs=wv_sb[:, e8, :], start=(e8 == 0), stop=(e8 == 7))
                nc.scalar.activation(
                    out=vhat[:, t16, :, 0:S],
                    in_=psv[:, 0:EPC].rearrange("p (h s) -> p h s", h=HPC),
                    func=mybir.ActivationFunctionType.Copy)

            # ---- LN apply via PE row-broadcast --------------------------
            # bp[:, 0:512] = a-row broadcast, bp[:, 512:1024] = c-row; the
            # selector lhsT lives at the same 32-aligned base as the rows.
            def ln_apply(src_t, m, c):
                sel = selrep_sb[32 * c:32 * c + 2, :]
                wcol = wb_sb[:, 0:1] if src_t is qt else wb_sb[:, 2:3]
                bcol = wb_sb[:, 1:2] if src_t is qt else wb_sb[:, 3:4]
                for ch in range(4):
                    sl = slice(ch * 512, (ch + 1) * 512)
                    bpa = psu1.tile([128, 512], FP32, tag="u1", name="bpa")
                    bpc = psu1.tile([128, 512], FP32, tag="u1", name="bpc")
                    nc.tensor.matmul(bpa, lhsT=sel,
                                     rhs=a_bfrows[32 * c:32 * c + 2, sl],
                                     start=True, stop=True,
                                     tile_position=(32 * c, 0))
                    nc.tensor.matmul(bpc, lhsT=sel,
                                     rhs=c_bfrows[32 * c:32 * c + 2, sl],
                                     start=True, stop=True,
                                     tile_position=(32 * c, 0))
                    nc.vector.tensor_tensor(out=src_t[m][:, sl], in0=src_t[m][:, sl],
                                            in1=bpa,
                                            op=mybir.AluOpType.mult)
                    nc.vector.tensor_tensor(out=src_t[m][:, sl], in0=src_t[m][:, sl],
                                            in1=bpc,
                                            op=mybir.AluOpType.subtract)
                nc.vector.tensor_scalar(out=src_t[m], in0=src_t[m],
                                        scalar1=wcol, scalar2=bcol,
                                        op0=mybir.AluOpType.mult,
                                        op1=mybir.AluOpType.add)

            # ---- attention (two head-pair streams interleaved) -----------
            ln_apply(qt, 0, 0)
            ln_apply(kt, 0, 2)
            ln_apply(qt, 1, 1)
            ln_apply(kt, 1, 3)

            def finish_norm(state):
                m_, qb_, otsb_, rcs_ = state
                nb = psu1.tile([128, 512], FP32, tag="u1", name="nb")
                nc.tensor.matmul(nb, lhsT=selh0_sb, rhs=rcs_[0],
                                 start=True, stop=False)
                nc.tensor.matmul(nb, lhsT=selh1_sb, rhs=rcs_[1],
                                 start=False, stop=True)
                rb = rbp.tile([128, 512], FP32, tag="rb")
                nc.vector.reciprocal(out=rb, in_=nb)
                nc.vector.tensor_tensor(
                    out=otb[m_][:, qb_ * 512:(qb_ + 1) * 512],
                    in0=otsb_, in1=rb, op=mybir.AluOpType.mult)

            def emit_wo(t16):
                pso = psu.tile([128, 1024], FP32, tag="u", name="pso")
                for e2 in range(2):
                    for mm in range(2):
                        nc.tensor.matmul(
                            pso[:, e2 * 512:(e2 + 1) * 512],
                            lhsT=otb[mm][:, t16 * 128:(t16 + 1) * 128],
                            rhs=wo_sb[:, mm, e2 * 512:(e2 + 1) * 512],
                            start=(mm == 0), stop=(mm == 1))
                osb = outp.tile([128, 1024], FP32, tag="osb")
                if t16 % 2 == 0:
                    nc.vector.tensor_copy(out=osb, in_=pso)
                else:
                    nc.scalar.activation(out=osb, in_=pso,
                                         func=mybir.ActivationFunctionType.Copy)
                nc.sync.dma_start(
                    out=out_e.ap()[t16 * 128:(t16 + 1) * 128, :],
                    in_=osb)

            def attn_stream(m):
                for qb in range(4):
                    otps = [psu1.tile([128, 512], FP32, tag="u1",
                                      name=f"otp{m}{h_}") for h_ in range(2)]
                    nkb = 4 * qb + 4
                    exq = []
                    for kb in range(nkb):
                        st = psu.tile([128, 1024], FP32, tag="u", name="st")
                        for h in range(2):
                            pa = slice(64 * h, 64 * h + 64)
                            nc.tensor.matmul(
                                st[:, h * 512:(h + 1) * 512],
                                lhsT=kt[m][pa, kb * 128:(kb + 1) * 128],
                                rhs=qt[m][pa, qb * 512:(qb + 1) * 512],
                                start=True, stop=True)
                        ex = expp.tile([128, 1024], BF16, tag="exp")
                        nc.scalar.activation(
                            out=ex, in_=st,
                            func=mybir.ActivationFunctionType.Exp)
                        d = kb - 4 * qb
                        if d >= 0:  # diagonal block: causal 0/1 mask
                            nc.vector.tensor_tensor(
                                out=ex, in0=ex, in1=masks_sb[:, d, :],
                                op=mybir.AluOpType.mult)
                        exq.append((ex, kb))
                        if len(exq) > 3:
                            exp_, kb_ = exq.pop(0)
                            for h in range(2):
                                nc.tensor.matmul(
                                    otps[h][0:S + 1, :],
                                    lhsT=vhat[:, kb_, 2 * m + h, :],
                                    rhs=exp_[:, h * 512:(h + 1) * 512],
                                    start=(kb_ == 0), stop=False)
                        yield None
                    while exq:
                        exp_, kb_ = exq.pop(0)
                        for h in range(2):
                            nc.tensor.matmul(
                                otps[h][0:S + 1, :],
                                lhsT=vhat[:, kb_, 2 * m + h, :],
                                rhs=exp_[:, h * 512:(h + 1) * 512],
                                start=(kb_ == 0), stop=(kb_ == nkb - 1))
                    # evict O^T + sums rows, then finish normalize on-chip
                    otsb = otsbp.tile([128, 512], FP32, tag="otsb")
                    rcs = []
                    for h in range(2):
                        rc = rcpp.tile([1, 512], BF16, tag="rc", name=f"rc{h}")
                        nc.scalar.activation(out=rc, in_=otps[h][S:S + 1, :],
                                             func=mybir.ActivationFunctionType.Copy)
                        nc.scalar.activation(out=otsb[64 * h:64 * h + 64, :],
                                             in_=otps[h][0:S, :],
                                             func=mybir.ActivationFunctionType.Copy)
                        rcs.append(rc)
                    finish_norm((m, qb, otsb, rcs))
                    yield qb

            g0, g1 = attn_stream(0), attn_stream(1)
            next(g0)
            next(g0)
            done0 = done1 = False
            while not (done0 and done1):
                if not done0:
                    try:
                        next(g0)
                    except StopIteration:
                        done0 = True
                if not done1:
                    try:
                        next(g1)
                    except StopIteration:
                        done1 = True
            for t16 in range(16):
                emit_wo(t16)
    return nc




# revision 56
# speedup vs baseline: 1.0469x; 1.0048x over previous
"""Multi-head attention with QK-LayerNorm on 8 TRN2 NeuronCores.

Shapes: B=2, T=2048, E=1024, H=16 heads, S=64 head dim.
Sharding: core c handles batch c//4 and the 4 heads [ (c%4)*4 , (c%4)*4+4 ).
Each core computes a partial output (its heads' contribution through Wo);
the host sums the 4 partials per batch and adds bo.

Device-side layout: activations are kept transposed ([feature, t]) so every
matmul contracts over the partition axis without on-device transposes:
  QT/KT   [s(64)*2heads = 128p, T]   (2 tiles per core, 2 heads each)
  V       [t 128p, head, s+1]        (extra ones-column -> softmax row sums)
  scores  S^T [t_k 128p, t_q 512]    (strictly-causal upper blocks skipped)
LayerNorm over s (the partition axis of QT) is done via matmul statistics
(block-diagonal ones lhsT), row math on [16, T] tiles, and a DRAM-roundtrip
partition-broadcast of the per-(head,t) scale/shift rows.
Softmax needs no max-subtraction: LN bounds logits to |q.k| <= ~2.
"""

import json
import math

import numpy as np
import ml_dtypes

import concourse.bass as bass
import concourse.bass2jax as bass2jax
import concourse.bass_utils as bass_utils
import concourse.tile as tile
from concourse import mybir
from concourse.vector_clock import ScopedClock

B, T, E, H, S = 2, 2048, 1024, 16, 64
HPC = 4            # heads per core
EPC = HPC * S      # feature cols per core = 256
LN_EPS = 1e-5
INV4 = float(E) ** -0.25
FP32 = mybir.dt.float32
BF16 = mybir.dt.bfloat16
BF = ml_dtypes.bfloat16

# ---------------------------------------------------------------------------
# Compile hook: this toolchain's walrus accepts at most ONE semaphore wait per
# TPB instruction. Tile attaches several. Split extras into standalone
# EventSemaphore (wait-only) instructions on the same engine.
# ---------------------------------------------------------------------------
_TPB_ENGINES = ("Pool", "Activation", "PE", "DVE", "SP")


def _split_multiwaits(bir_json: bytes) -> bytes:
    d = json.loads(bir_json)
    n_split = 0
    for fn in d.get("functions", []):
        for blk in fn.get("blocks", []):
            insts = blk.get("instructions", [])
            out = []
            for inst in insts:
                si = inst.get("sync_info")
                waits = (si or {}).get("on_wait") or []
                if si and len(waits) > 1 and inst.get("engine") in _TPB_ENGINES:
                    for i, w in enumerate(waits[:-1]):
                        out.append({
                            "debug": inst.get("debug", 0),
                            "engine": inst["engine"],
                            "ins": [],
                            "name": f"{inst['name']}-ws{i}",
                            "opcode": "EventSemaphore",
                            "outs": [],
                            "sync_info": {"on_update": [], "on_wait": [w]},
                        })
                        n_split += 1
                    si["on_wait"] = [waits[-1]]
                out.append(inst)
            blk["instructions"] = out
    return json.dumps(d).encode()


_orig_compile_bir_kernel = bass_utils.compile_bir_kernel


def _patched_compile_bir_kernel(bir_json, tmpdir, neff_name="file.neff"):
    return _orig_compile_bir_kernel(_split_multiwaits(bir_json), tmpdir, neff_name)



bass_utils.compile_bir_kernel = _patched_compile_bir_kernel
bass2jax.compile_bir_kernel = _patched_compile_bir_kernel


def _patched_drain_and_barrier(self, tick_clock, wait_clock):
    # Same as TileContext._drain_and_barrier but the drain's waits are emitted
    # as single-wait instructions (walrus limit).
    gc = tick_clock.global_clock
    ticks = eval(str(gc).replace("VectorClock(", "").rstrip(")"))
    sems = wait_clock.sems.allocated()
    for proc_idx, sem in sems.items():
        t = ticks[proc_idx]
        if t > 0:
            mult = 16 if proc_idx >= 11 else 1
            self.nc.sync.wait_ge(sem, t * mult)
    self.nc.sync.drain()
    self.nc.all_engine_barrier()
    assert self.sems is not None
    popped = self.nc._tile_sem_poison_stack.pop()
    assert popped is self._sem_poison
    self.nc.clear_and_free_semaphores(list(self.sems.allocated().values()))
    self.nc.all_engine_barrier()


tile.TileContext._drain_and_barrier = _patched_drain_and_barrier


# ---------------------------------------------------------------------------
# Device kernel (identical program on all 8 cores)
# ---------------------------------------------------------------------------


def _act_raw(nc, out, in_, func):
    # nc.scalar.activation refuses Reciprocal (accuracy); our tolerance is
    # 2e-2 so the LUT version is fine. Emit InstActivation directly.
    eng = nc.scalar
    inputs = [eng.lower_ap(in_)]
    for arg in (0.0, 1.0, 0.0):  # bias, scale, alpha
        inputs.append(mybir.ImmediateValue(dtype=mybir.dt.float32, value=arg))
    return eng.add_instruction(
        mybir.InstActivation(
            name=nc.get_next_instruction_name(),
            func=func,
            ins=inputs,
            outs=[eng.lower_ap(out)],
        )
    )


def _build_bass():
    nc = bass.Bass()
    xtq_e = nc.dram_tensor("xtq", [128, 8, T], BF16, kind="ExternalInput")
    xtk_e = nc.dram_tensor("xtk", [128, 8, T], BF16, kind="ExternalInput")
    xtv_e = nc.dram_tensor("xtv", [128, 8, T], BF16, kind="ExternalInput")
    wq_e = nc.dram_tensor("wq", [128, 8, EPC], BF16, kind="ExternalInput")
    wk_e = nc.dram_tensor("wk", [128, 8, EPC], BF16, kind="ExternalInput")
    wv_e = nc.dram_tensor("wv", [128, 8, EPC], BF16, kind="ExternalInput")
    wo_e = nc.dram_tensor("wo", [128, 2, E], BF16, kind="ExternalInput")
    masks_e = nc.dram_tensor("masks", [128, 4, 1024], BF16, kind="ExternalInput")
    eye_e = nc.dram_tensor("eye2", [128, 2], BF16, kind="ExternalInput")
    wb_e = nc.dram_tensor("wbcols", [128, 4], FP32, kind="ExternalInput")
    selrep_e = nc.dram_tensor("selrep", [128, 128], BF16, kind="ExternalInput")
    selh_e = nc.dram_tensor("selh", [2, 128], BF16, kind="ExternalInput")
    out_e = nc.dram_tensor("out", [T, E], FP32, kind="ExternalOutput")

    xtq, xtk, xtv = xtq_e.ap(), xtk_e.ap(), xtv_e.ap()
    wq_a, wk_a, wv_a, wo_a = wq_e.ap(), wk_e.ap(), wv_e.ap(), wo_e.ap()

    with tile.TileContext(nc) as tc:
        with tc.tile_pool(name="singles", bufs=1) as singles, \
             tc.tile_pool(name="xstream", bufs=8) as xstream, \
             tc.tile_pool(name="work", bufs=1) as work, \
             tc.tile_pool(name="rows", bufs=1) as rows, \
             tc.tile_pool(name="expp", bufs=10) as expp, \
             tc.tile_pool(name="outp", bufs=3) as outp, \
             tc.tile_pool(name="otsbp", bufs=2) as otsbp, \
             tc.tile_pool(name="rcp", bufs=6) as rcpp, \
             tc.tile_pool(name="rbp", bufs=2) as rbp, \
             tc.tile_pool(name="psu", bufs=2, space="PSUM") as psu, \
             tc.tile_pool(name="psu1", bufs=4, space="PSUM") as psu1:

            # ---- resident constants (issue order = DMA priority) ---------
            wq_sb = singles.tile([128, 8, EPC], BF16)
            wk_sb = singles.tile([128, 8, EPC], BF16)
            eye_sb = singles.tile([128, 2], BF16)
            nc.scalar.dma_start(out=eye_sb, in_=eye_e.ap())
            wb_sb = singles.tile([128, 4], FP32)
            nc.scalar.dma_start(out=wb_sb, in_=wb_e.ap())
            selrep_sb = singles.tile([128, 128], BF16)
            nc.scalar.dma_start(out=selrep_sb, in_=selrep_e.ap())
            selh0_sb = singles.tile([1, 128], BF16)
            nc.scalar.dma_start(out=selh0_sb, in_=selh_e.ap()[0:1, :])
            selh1_sb = singles.tile([1, 128], BF16)
            nc.scalar.dma_start(out=selh1_sb, in_=selh_e.ap()[1:2, :])
            xtv_sb = singles.tile([128, 8, T], BF16)
            wv_sb = singles.tile([128, 8, EPC], BF16)
            masks_sb = singles.tile([128, 4, 1024], BF16)
            wo_sb = singles.tile([128, 2, E], BF16)

            qt = [singles.tile([128, T], BF16, tag=f"qt{m}", name=f"qt{m}") for m in range(2)]
            kt = [singles.tile([128, T], BF16, tag=f"kt{m}", name=f"kt{m}") for m in range(2)]
            vhat = singles.tile([128, 16, HPC, S + 1], BF16)
            otb = [singles.tile([128, T], BF16, tag=f"otb{m}", name=f"otb{m}") for m in range(2)]
            nc.vector.memset(vhat[:, :, :, S:S + 1], 1.0)

            # ---- Q/K projections + LN statistics (interleaved) -----------
            sums_t = rows.tile([128, T], FP32)
            sumsq_t = rows.tile([128, T], FP32)

            def ln_stats(src_t, m, c):
                sq = work.tile([128, T], BF16, tag="sq")
                nc.vector.tensor_tensor(out=sq, in0=src_t[m], in1=src_t[m],
                                        op=mybir.AluOpType.mult)
                for n in range(4):
                    sl = slice(n * 512, (n + 1) * 512)
                    ps_s = psu1.tile([128, 512], FP32, tag="u1", name="st_s")
                    ps_q = psu1.tile([128, 512], FP32, tag="u1", name="st_q")
                    nc.tensor.matmul(ps_s[0:2, :], lhsT=eye_sb, rhs=src_t[m][:, sl],
                                     start=True, stop=True)
                    nc.tensor.matmul(ps_q[0:2, :], lhsT=eye_sb, rhs=sq[:, sl],
                                     start=True, stop=True)
                    if n % 2 == 0:
                        nc.scalar.activation(out=sums_t[32 * c:32 * c + 2, sl],
                                             in_=ps_s[0:2, :],
                                             func=mybir.ActivationFunctionType.Copy)
                        nc.scalar.activation(out=sumsq_t[32 * c:32 * c + 2, sl],
                                             in_=ps_q[0:2, :],
                                             func=mybir.ActivationFunctionType.Copy)
                    else:
                        nc.vector.tensor_copy(out=sums_t[32 * c:32 * c + 2, sl],
                                              in_=ps_s[0:2, :])
                        nc.vector.tensor_copy(out=sumsq_t[32 * c:32 * c + 2, sl],
                                              in_=ps_q[0:2, :])

            for qk_i, (x_ap, w_sb, dst) in enumerate(((xtq, wq_sb, qt), (xtk, wk_sb, kt))):
                if qk_i == 1:
                    nc.sync.dma_start(out=wk_sb, in_=wk_a)
                xcs = {}
                for m in range(2):
                    if qk_i == 1 and m == 1:
                        nc.sync.dma_start(out=wv_sb, in_=wv_a)
                        nc.sync.dma_start(out=masks_sb, in_=masks_e.ap())
                        nc.sync.dma_start(out=wo_sb, in_=wo_a)
                    pss = [psu.tile([128, 1024], FP32, tag="u", name=f"pss{j}")
                           for j in range(2)]
                    for e8 in range(8):
                        if m == 0:
                            if qk_i == 0:
                                nc.sync.dma_start(out=wq_sb[:, e8, :],
                                                  in_=wq_a[:, e8, :])
                            xc = xstream.tile([128, T], BF16, tag="xchunk",
                                              name=f"xc{e8}")
                            nc.sync.dma_start(out=xc, in_=x_ap[:, e8, :])
                            if qk_i == 1:
                                nc.sync.dma_start(out=xtv_sb[:, e8, :],
                                                  in_=xtv[:, e8, :])
                            xcs[e8] = xc
                        xc = xcs[e8]
                        for n in range(4):
                            nc.tensor.matmul(
                                pss[n // 2][:, (n % 2) * 512:(n % 2) * 512 + 512],
                                lhsT=w_sb[:, e8, m * 128:(m + 1) * 128],
                                rhs=xc[:, n * 512:(n + 1) * 512],
                                start=(e8 == 0), stop=(e8 == 7))
                    for j in range(2):
                        nc.vector.tensor_copy(
                            out=dst[m][:, j * 1024:(j + 1) * 1024], in_=pss[j])
                    ln_stats(dst, m, 2 * qk_i + m)

            # ---- LN row math (overlaps V projection) --------------------
            eps_col = singles.tile([128, 1], FP32)
            nc.vector.memset(eps_col, LN_EPS)
            nc.vector.tensor_scalar_mul(sums_t, sums_t, 1.0 / S)          # mu
            nc.vector.tensor_scalar_mul(sumsq_t, sumsq_t, 1.0 / S)
            tmp = rows.tile([128, T], FP32)
            nc.vector.tensor_tensor(out=tmp, in0=sums_t, in1=sums_t,
                                    op=mybir.AluOpType.mult)
            nc.vector.tensor_tensor(out=sumsq_t, in0=sumsq_t, in1=tmp,
                                    op=mybir.AluOpType.subtract)
            nc.vector.tensor_scalar_max(sumsq_t, sumsq_t, 0.0)
            nc.scalar.activation(out=sumsq_t, in_=sumsq_t,
                                 func=mybir.ActivationFunctionType.Sqrt,
                                 bias=eps_col)
            _act_raw(nc, sumsq_t, sumsq_t,
                     mybir.ActivationFunctionType.Reciprocal)             # rstd
            nc.vector.tensor_tensor(out=tmp, in0=sums_t, in1=sumsq_t,
                                    op=mybir.AluOpType.mult)              # mu*rstd
            c_bfrows = rows.tile([128, T], BF16)
            a_bfrows = rows.tile([128, T], BF16)
            nc.vector.tensor_copy(out=c_bfrows, in_=tmp)
            nc.vector.tensor_copy(out=a_bfrows, in_=sumsq_t)

            # ---- V projection (natural layout + ones column) -------------
            for t16 in range(16):
                psv = psu.tile([128, 1024], FP32, tag="u", name="psv")
                for e8 in range(8):
                    nc.tensor.matmul(
                        psv[:, 0:EPC], lhsT=xtv_sb[:, e8, t16 * 128:(t16 + 1) * 128],
                        rhs=wv_sb[:, e8, :], start=(e8 == 0), stop=(e8 == 7))
                nc.scalar.activation(
                    out=vhat[:, t16, :, 0:S],
                    in_=psv[:, 0:EPC].rearrange("p (h s) -> p h s", h=HPC),
                    func=mybir.ActivationFunctionType.Copy)

            # ---- LN apply via PE row-broadcast --------------------------
            # bp[:, 0:512] = a-row broadcast, bp[:, 512:1024] = c-row; the
            # selector lhsT lives at the same 32-aligned base as the rows.
            def ln_apply(src_t, m, c):
                sel = selrep_sb[32 * c:32 * c + 2, :]
                wcol = wb_sb[:, 0:1] if src_t is qt else wb_sb[:, 2:3]
                bcol = wb_sb[:, 1:2] if src_t is qt else wb_sb[:, 3:4]
                for ch in range(4):
                    sl = slice(ch * 512, (ch + 1) * 512)
                    bpa = psu1.tile([128, 512], FP32, tag="u1", name="bpa")
                    bpc = psu1.tile([128, 512], FP32, tag="u1", name="bpc")
                    nc.tensor.matmul(bpa, lhsT=sel,
                                     rhs=a_bfrows[32 * c:32 * c + 2, sl],
                                     start=True, stop=True,
                                     tile_position=(32 * c, 0))
                    nc.tensor.matmul(bpc, lhsT=sel,
                                     rhs=c_bfrows[32 * c:32 * c + 2, sl],
                                     start=True, stop=True,
                                     tile_position=(32 * c, 0))
                    nc.vector.tensor_tensor(out=src_t[m][:, sl], in0=src_t[m][:, sl],
                                            in1=bpa,
                                            op=mybir.AluOpType.mult)
                    nc.vector.tensor_tensor(out=src_t[m][:, sl], in0=src_t[m][:, sl],
                                            in1=bpc,
                                            op=mybir.AluOpType.subtract)
                nc.vector.tensor_scalar(out=src_t[m], in0=src_t[m],
                                        scalar1=wcol, scalar2=bcol,
                                        op0=mybir.AluOpType.mult,
                                        op1=mybir.AluOpType.add)

            # ---- attention (two head-pair streams interleaved) -----------
            ln_apply(qt, 0, 0)
            ln_apply(kt, 0, 2)
            ln_apply(qt, 1, 1)
            ln_apply(kt, 1, 3)

            def finish_norm(state):
                m_, qb_, otsb_, rcs_ = state
                nb = psu1.tile([128, 512], FP32, tag="u1", name="nb")
                nc.tensor.matmul(nb, lhsT=selh0_sb, rhs=rcs_[0],
                                 start=True, stop=False)
                nc.tensor.matmul(nb, lhsT=selh1_sb, rhs=rcs_[1],
                                 start=False, stop=True)
                rb = rbp.tile([128, 512], FP32, tag="rb")
                nc.vector.reciprocal(out=rb, in_=nb)
                nc.vector.tensor_tensor(
                    out=otb[m_][:, qb_ * 512:(qb_ + 1) * 512],
                    in0=otsb_, in1=rb, op=mybir.AluOpType.mult)

            def emit_wo(t16):
                pso = psu.tile([128, 1024], FP32, tag="u", name="pso")
                for e2 in range(2):
                    for mm in range(2):
                        nc.tensor.matmul(
                            pso[:, e2 * 512:(e2 + 1) * 512],
                            lhsT=otb[mm][:, t16 * 128:(t16 + 1) * 128],
                            rhs=wo_sb[:, mm, e2 * 512:(e2 + 1) * 512],
                            start=(mm == 0), stop=(mm == 1))
                osb = outp.tile([128, 1024], FP32, tag="osb")
                if t16 % 2 == 0:
                    nc.vector.tensor_copy(out=osb, in_=pso)
                else:
                    nc.scalar.activation(out=osb, in_=pso,
                                         func=mybir.ActivationFunctionType.Copy)
                nc.sync.dma_start(
                    out=out_e.ap()[t16 * 128:(t16 + 1) * 128, :],
                    in_=osb)

            def attn_stream(m):
                for qb in range(4):
                    otps = [psu1.tile([128, 512], FP32, tag="u1",
                                      name=f"otp{m}{h_}") for h_ in range(2)]
                    nkb = 4 * qb + 4
                    exq = []
                    for kb in range(nkb):
                        st = psu.tile([128, 1024], FP32, tag="u", name="st")
                        for h in range(2):
                            pa = slice(64 * h, 64 * h + 64)
                            nc.tensor.matmul(
                                st[:, h * 512:(h + 1) * 512],
                                lhsT=kt[m][pa, kb * 128:(kb + 1) * 128],
                                rhs=qt[m][pa, qb * 512:(qb + 1) * 512],
                                start=True, stop=True)
                        ex = expp.tile([128, 1024], BF16, tag="exp")
                        nc.scalar.activation(
                            out=ex, in_=st,
                            func=mybir.ActivationFunctionType.Exp)
                        d = kb - 4 * qb
                        if d >= 0:  # diagonal block: causal 0/1 mask
                            nc.vector.tensor_tensor(
                                out=ex, in0=ex, in1=masks_sb[:, d, :],
                                op=mybir.AluOpType.mult)
                        exq.append((ex, kb))
                        if len(exq) > 3:
                            exp_, kb_ = exq.pop(0)
                            for h in range(2):
                                nc.tensor.matmul(
                                    otps[h][0:S + 1, :],
                                    lhsT=vhat[:, kb_, 2 * m + h, :],
                                    rhs=exp_[:, h * 512:(h + 1) * 512],
                                    start=(kb_ == 0), stop=False)
                        yield None
                    while exq:
                        exp_, kb_ = exq.pop(0)
                        for h in range(2):
                            nc.tensor.matmul(
                                otps[h][0:S + 1, :],
                                lhsT=vhat[:, kb_, 2 * m + h, :],
                                rhs=exp_[:, h * 512:(h + 1) * 512],
                                start=(kb_ == 0), stop=(kb_ == nkb - 1))
                    # evict O^T + sums rows, then finish normalize on-chip
                    otsb = otsbp.tile([128, 512], FP32, tag="otsb")
                    rcs = []
                    for h in range(2):
                        rc = rcpp.tile([1, 512], BF16, tag="rc", name=f"rc{h}")
                        nc.scalar.activation(out=rc, in_=otps[h][S:S + 1, :],
                                             func=mybir.ActivationFunctionType.Copy)
                        nc.scalar.activation(out=otsb[64 * h:64 * h + 64, :],
                                             in_=otps[h][0:S, :],
                                             func=mybir.ActivationFunctionType.Copy)
                        rcs.append(rc)
                    finish_norm((m, qb, otsb, rcs))
                    yield qb

            g0, g1 = attn_stream(0), attn_stream(1)
            next(g0)
            next(g0)
            done0 = done1 = False
            while not (done0 and done1):
                if not done0:
                    try:
                        next(g0)
                    except StopIteration:
                        done0 = True
                if not done1:
                    try:
                        next(g1)
                    except StopIteration:
                        done1 = True
            for t16 in range(16):
                emit_wo(t16)
    return nc




# revision 57
# speedup vs baseline: 1.0654x; 1.0177x over previous
"""Multi-head attention with QK-LayerNorm on 8 TRN2 NeuronCores.

Shapes: B=2, T=2048, E=1024, H=16 heads, S=64 head dim.
Sharding: core c handles batch c//4 and the 4 heads [ (c%4)*4 , (c%4)*4+4 ).
Each core computes a partial output (its heads' contribution through Wo);
the host sums the 4 partials per batch and adds bo.

Device-side layout: activations are kept transposed ([feature, t]) so every
matmul contracts over the partition axis without on-device transposes:
  QT/KT   [s(64)*2heads = 128p, T]   (2 tiles per core, 2 heads each)
  V       [t 128p, head, s+1]        (extra ones-column -> softmax row sums)
  scores  S^T [t_k 128p, t_q 512]    (strictly-causal upper blocks skipped)
LayerNorm over s (the partition axis of QT) is done via matmul statistics
(block-diagonal ones lhsT), row math on [16, T] tiles, and a DRAM-roundtrip
partition-broadcast of the per-(head,t) scale/shift rows.
Softmax needs no max-subtraction: LN bounds logits to |q.k| <= ~2.
"""

import json

import numpy as np
import ml_dtypes

import concourse.bass as bass
import concourse.bass2jax as bass2jax
import concourse.bass_utils as bass_utils
import concourse.tile as tile
from concourse import mybir

B, T, E, H, S = 2, 2048, 1024, 16, 64
HPC = 4            # heads per core
EPC = HPC * S      # feature cols per core = 256
LN_EPS = 1e-5
INV4 = float(E) ** -0.25
FP32 = mybir.dt.float32
BF16 = mybir.dt.bfloat16
BF = ml_dtypes.bfloat16

# ---------------------------------------------------------------------------
# Compile hook: this toolchain's walrus accepts at most ONE semaphore wait per
# TPB instruction. Tile attaches several. Split extras into standalone
# EventSemaphore (wait-only) instructions on the same engine.
# ---------------------------------------------------------------------------
_TPB_ENGINES = ("Pool", "Activation", "PE", "DVE", "SP")


def _split_multiwaits(bir_json: bytes) -> bytes:
    d = json.loads(bir_json)
    n_split = 0
    for fn in d.get("functions", []):
        for blk in fn.get("blocks", []):
            insts = blk.get("instructions", [])
            out = []
            for inst in insts:
                si = inst.get("sync_info")
                waits = (si or {}).get("on_wait") or []
                if si and len(waits) > 1 and inst.get("engine") in _TPB_ENGINES:
                    for i, w in enumerate(waits[:-1]):
                        out.append({
                            "debug": inst.get("debug", 0),
                            "engine": inst["engine"],
                            "ins": [],
                            "name": f"{inst['name']}-ws{i}",
                            "opcode": "EventSemaphore",
                            "outs": [],
                            "sync_info": {"on_update": [], "on_wait": [w]},
                        })
                        n_split += 1
                    si["on_wait"] = [waits[-1]]
                out.append(inst)
            blk["instructions"] = out
    return json.dumps(d).encode()


_orig_compile_bir_kernel = bass_utils.compile_bir_kernel


def _patched_compile_bir_kernel(bir_json, tmpdir, neff_name="file.neff"):
    return _orig_compile_bir_kernel(_split_multiwaits(bir_json), tmpdir, neff_name)



bass_utils.compile_bir_kernel = _patched_compile_bir_kernel
bass2jax.compile_bir_kernel = _patched_compile_bir_kernel


def _patched_drain_and_barrier(self, tick_clock, wait_clock):
    # Same as TileContext._drain_and_barrier but the drain's waits are emitted
    # as single-wait instructions (walrus limit).
    gc = tick_clock.global_clock
    ticks = eval(str(gc).replace("VectorClock(", "").rstrip(")"))
    sems = wait_clock.sems.allocated()
    for proc_idx, sem in sems.items():
        t = ticks[proc_idx]
        if t > 0:
            mult = 16 if proc_idx >= 11 else 1
            self.nc.sync.wait_ge(sem, t * mult)
    self.nc.sync.drain()
    self.nc.all_engine_barrier()
    assert self.sems is not None
    popped = self.nc._tile_sem_poison_stack.pop()
    assert popped is self._sem_poison
    self.nc.clear_and_free_semaphores(list(self.sems.allocated().values()))
    self.nc.all_engine_barrier()


tile.TileContext._drain_and_barrier = _patched_drain_and_barrier


# ---------------------------------------------------------------------------
# Device kernel (identical program on all 8 cores)
# ---------------------------------------------------------------------------


def _act_raw(nc, out, in_, func):
    # nc.scalar.activation refuses Reciprocal (accuracy); our tolerance is
    # 2e-2 so the LUT version is fine. Emit InstActivation directly.
    eng = nc.scalar
    inputs = [eng.lower_ap(in_)]
    for arg in (0.0, 1.0, 0.0):  # bias, scale, alpha
        inputs.append(mybir.ImmediateValue(dtype=mybir.dt.float32, value=arg))
    return eng.add_instruction(
        mybir.InstActivation(
            name=nc.get_next_instruction_name(),
            func=func,
            ins=inputs,
            outs=[eng.lower_ap(out)],
        )
    )


def _build_bass():
    nc = bass.Bass()
    xtq_e = nc.dram_tensor("xtq", [128, 8, T], BF16, kind="ExternalInput")
    xtk_e = nc.dram_tensor("xtk", [128, 8, T], BF16, kind="ExternalInput")
    xtv_e = nc.dram_tensor("xtv", [128, 8, T], BF16, kind="ExternalInput")
    wq_e = nc.dram_tensor("wq", [128, 8, EPC], BF16, kind="ExternalInput")
    wk_e = nc.dram_tensor("wk", [128, 8, EPC], BF16, kind="ExternalInput")
    wv_e = nc.dram_tensor("wv", [128, 8, EPC], BF16, kind="ExternalInput")
    wo_e = nc.dram_tensor("wo", [128, 2, E], BF16, kind="ExternalInput")
    masks_e = nc.dram_tensor("masks", [128, 4, 1024], BF16, kind="ExternalInput")
    eye_e = nc.dram_tensor("eye2", [128, 2], BF16, kind="ExternalInput")
    wb_e = nc.dram_tensor("wbcols", [128, 4], FP32, kind="ExternalInput")
    selrep_e = nc.dram_tensor("selrep", [128, 128], BF16, kind="ExternalInput")
    selh_e = nc.dram_tensor("selh", [2, 128], BF16, kind="ExternalInput")
    out_e = nc.dram_tensor("out", [T, E], FP32, kind="ExternalOutput")

    xtq, xtk, xtv = xtq_e.ap(), xtk_e.ap(), xtv_e.ap()
    wq_a, wk_a, wv_a, wo_a = wq_e.ap(), wk_e.ap(), wv_e.ap(), wo_e.ap()

    with tile.TileContext(nc) as tc:
        with tc.tile_pool(name="singles", bufs=1) as singles, \
             tc.tile_pool(name="xstream", bufs=8) as xstream, \
             tc.tile_pool(name="work", bufs=1) as work, \
             tc.tile_pool(name="rows", bufs=1) as rows, \
             tc.tile_pool(name="expp", bufs=10) as expp, \
             tc.tile_pool(name="outp", bufs=3) as outp, \
             tc.tile_pool(name="otsbp", bufs=2) as otsbp, \
             tc.tile_pool(name="rcp", bufs=6) as rcpp, \
             tc.tile_pool(name="rbp", bufs=2) as rbp, \
             tc.tile_pool(name="psu", bufs=2, space="PSUM") as psu, \
             tc.tile_pool(name="psu1", bufs=4, space="PSUM") as psu1:

            # ---- resident constants (issue order = DMA priority) ---------
            wq_sb = singles.tile([128, 8, EPC], BF16)
            wk_sb = singles.tile([128, 8, EPC], BF16)
            eye_sb = singles.tile([128, 2], BF16)
            nc.scalar.dma_start(out=eye_sb, in_=eye_e.ap())
            wb_sb = singles.tile([128, 4], FP32)
            nc.scalar.dma_start(out=wb_sb, in_=wb_e.ap())
            selrep_sb = singles.tile([128, 128], BF16)
            nc.scalar.dma_start(out=selrep_sb, in_=selrep_e.ap())
            selh0_sb = singles.tile([1, 128], BF16)
            nc.scalar.dma_start(out=selh0_sb, in_=selh_e.ap()[0:1, :])
            selh1_sb = singles.tile([1, 128], BF16)
            nc.scalar.dma_start(out=selh1_sb, in_=selh_e.ap()[1:2, :])
            xtv_sb = singles.tile([128, 8, T], BF16)
            wv_sb = singles.tile([128, 8, EPC], BF16)
            masks_sb = singles.tile([128, 4, 1024], BF16)
            wo_sb = singles.tile([128, 2, E], BF16)

            qt = [singles.tile([128, T], BF16, tag=f"qt{m}", name=f"qt{m}") for m in range(2)]
            kt = [singles.tile([128, T], BF16, tag=f"kt{m}", name=f"kt{m}") for m in range(2)]
            vhat = singles.tile([128, 16, HPC, S + 1], BF16)
            otb = [singles.tile([128, T], BF16, tag=f"otb{m}", name=f"otb{m}") for m in range(2)]
            nc.vector.memset(vhat[:, :, :, S:S + 1], 1.0)

            # ---- Q/K projections + LN statistics (interleaved) -----------
            sums_t = rows.tile([128, T], FP32)
            sumsq_t = rows.tile([128, T], FP32)

            def ln_stats(src_t, m, c):
                sq = work.tile([128, T], BF16, tag="sq")
                nc.vector.tensor_tensor(out=sq, in0=src_t[m], in1=src_t[m],
                                        op=mybir.AluOpType.mult)
                for n in range(4):
                    sl = slice(n * 512, (n + 1) * 512)
                    ps_s = psu1.tile([128, 512], FP32, tag="u1", name="st_s")
                    ps_q = psu1.tile([128, 512], FP32, tag="u1", name="st_q")
                    nc.tensor.matmul(ps_s[0:2, :], lhsT=eye_sb, rhs=src_t[m][:, sl],
                                     start=True, stop=True)
                    nc.tensor.matmul(ps_q[0:2, :], lhsT=eye_sb, rhs=sq[:, sl],
                                     start=True, stop=True)
                    if n % 2 == 0:
                        nc.scalar.activation(out=sums_t[32 * c:32 * c + 2, sl],
                                             in_=ps_s[0:2, :],
                                             func=mybir.ActivationFunctionType.Copy)
                        nc.scalar.activation(out=sumsq_t[32 * c:32 * c + 2, sl],
                                             in_=ps_q[0:2, :],
                                             func=mybir.ActivationFunctionType.Copy)
                    else:
                        nc.vector.tensor_copy(out=sums_t[32 * c:32 * c + 2, sl],
                                              in_=ps_s[0:2, :])
                        nc.vector.tensor_copy(out=sumsq_t[32 * c:32 * c + 2, sl],
                                              in_=ps_q[0:2, :])

            for qk_i, (x_ap, w_sb, dst) in enumerate(((xtq, wq_sb, qt), (xtk, wk_sb, kt))):
                if qk_i == 1:
                    nc.sync.dma_start(out=wk_sb, in_=wk_a)
                xcs = {}
                for m in range(2):
                    if qk_i == 1 and m == 1:
                        nc.sync.dma_start(out=wv_sb, in_=wv_a)
                        nc.sync.dma_start(out=masks_sb, in_=masks_e.ap())
                        nc.sync.dma_start(out=wo_sb, in_=wo_a)
                    pss = [psu.tile([128, 1024], FP32, tag="u", name=f"pss{j}")
                           for j in range(2)]
                    for e8 in range(8):
                        if m == 0:
                            if qk_i == 0:
                                nc.sync.dma_start(out=wq_sb[:, e8, :],
                                                  in_=wq_a[:, e8, :])
                            xc = xstream.tile([128, T], BF16, tag="xchunk",
                                              name=f"xc{e8}")
                            nc.sync.dma_start(out=xc, in_=x_ap[:, e8, :])
                            if qk_i == 1:
                                nc.sync.dma_start(out=xtv_sb[:, e8, :],
                                                  in_=xtv[:, e8, :])
                            xcs[e8] = xc
                        xc = xcs[e8]
                        for n in range(4):
                            nc.tensor.matmul(
                                pss[n // 2][:, (n % 2) * 512:(n % 2) * 512 + 512],
                                lhsT=w_sb[:, e8, m * 128:(m + 1) * 128],
                                rhs=xc[:, n * 512:(n + 1) * 512],
                                start=(e8 == 0), stop=(e8 == 7))
                    for j in range(2):
                        nc.vector.tensor_copy(
                            out=dst[m][:, j * 1024:(j + 1) * 1024], in_=pss[j])
                    ln_stats(dst, m, 2 * qk_i + m)

            # ---- LN row math (overlaps V projection) --------------------
            eps_col = singles.tile([128, 1], FP32)
            nc.vector.memset(eps_col, LN_EPS)
            nc.vector.tensor_scalar_mul(sums_t, sums_t, 1.0 / S)          # mu
            nc.vector.tensor_scalar_mul(sumsq_t, sumsq_t, 1.0 / S)
            tmp = rows.tile([128, T], FP32)
            nc.vector.tensor_tensor(out=tmp, in0=sums_t, in1=sums_t,
                                    op=mybir.AluOpType.mult)
            nc.vector.tensor_tensor(out=sumsq_t, in0=sumsq_t, in1=tmp,
                                    op=mybir.AluOpType.subtract)
            nc.vector.tensor_scalar_max(sumsq_t, sumsq_t, 0.0)
            nc.scalar.activation(out=sumsq_t, in_=sumsq_t,
                                 func=mybir.ActivationFunctionType.Sqrt,
                                 bias=eps_col)
            _act_raw(nc, sumsq_t, sumsq_t,
                     mybir.ActivationFunctionType.Reciprocal)             # rstd
            nc.vector.tensor_tensor(out=tmp, in0=sums_t, in1=sumsq_t,
                                    op=mybir.AluOpType.mult)              # mu*rstd
            c_bfrows = rows.tile([128, T], BF16)
            a_bfrows = rows.tile([128, T], BF16)
            nc.vector.tensor_copy(out=c_bfrows, in_=tmp)
            nc.vector.tensor_copy(out=a_bfrows, in_=sumsq_t)

            # ---- V projection (natural layout + ones column) -------------
            for t16 in range(16):
                psv = psu.tile([128, 1024], FP32, tag="u", name="psv")
                for e8 in range(8):
                    nc.tensor.matmul(
                        psv[:, 0:EPC], lhsT=xtv_sb[:, e8, t16 * 128:(t16 + 1) * 128],
                        rhs=wv_sb[:, e8, :], start=(e8 == 0), stop=(e8 == 7))
                nc.scalar.activation(
                    out=vhat[:, t16, :, 0:S],
                    in_=psv[:, 0:EPC].rearrange("p (h s) -> p h s", h=HPC),
                    func=mybir.ActivationFunctionType.Copy)

            # ---- LN apply via PE row-broadcast --------------------------
            # bp[:, 0:512] = a-row broadcast, bp[:, 512:1024] = c-row; the
            # selector lhsT lives at the same 32-aligned base as the rows.
            def ln_apply(src_t, m, c):
                sel = selrep_sb[32 * c:32 * c + 2, :]
                wcol = wb_sb[:, 0:1] if src_t is qt else wb_sb[:, 2:3]
                bcol = wb_sb[:, 1:2] if src_t is qt else wb_sb[:, 3:4]
                for ch in range(4):
                    sl = slice(ch * 512, (ch + 1) * 512)
                    bpa = psu1.tile([128, 512], FP32, tag="u1", name="bpa")
                    bpc = psu1.tile([128, 512], FP32, tag="u1", name="bpc")
                    nc.tensor.matmul(bpa, lhsT=sel,
                                     rhs=a_bfrows[32 * c:32 * c + 2, sl],
                                     start=True, stop=True,
                                     tile_position=(32 * c, 0))
                    nc.tensor.matmul(bpc, lhsT=sel,
                                     rhs=c_bfrows[32 * c:32 * c + 2, sl],
                                     start=True, stop=True,
                                     tile_position=(32 * c, 0))
                    nc.vector.tensor_tensor(out=src_t[m][:, sl], in0=src_t[m][:, sl],
                                            in1=bpa,
                                            op=mybir.AluOpType.mult)
                    nc.vector.tensor_tensor(out=src_t[m][:, sl], in0=src_t[m][:, sl],
                                            in1=bpc,
                                            op=mybir.AluOpType.subtract)
                nc.vector.tensor_scalar(out=src_t[m], in0=src_t[m],
                                        scalar1=wcol, scalar2=bcol,
                                        op0=mybir.AluOpType.mult,
                                        op1=mybir.AluOpType.add)

            # ---- attention (two head-pair streams interleaved) -----------
            ln_apply(qt, 0, 0)
            ln_apply(kt, 0, 2)
            ln_apply(qt, 1, 1)
            ln_apply(kt, 1, 3)

            def finish_norm(state):
                m_, qb_, otsb_, rcs_ = state
                nb = psu1.tile([128, 512], FP32, tag="u1", name="nb")
                nc.tensor.matmul(nb, lhsT=selh0_sb, rhs=rcs_[0],
                                 start=True, stop=False)
                nc.tensor.matmul(nb, lhsT=selh1_sb, rhs=rcs_[1],
                                 start=False, stop=True)
                rb = rbp.tile([128, 512], FP32, tag="rb")
                nc.vector.reciprocal(out=rb, in_=nb)
                nc.vector.tensor_tensor(
                    out=otb[m_][:, qb_ * 512:(qb_ + 1) * 512],
                    in0=otsb_, in1=rb, op=mybir.AluOpType.mult)

            def emit_wo(t16):
                pso = psu.tile([128, 1024], FP32, tag="u", name="pso")
                for e2 in range(2):
                    for mm in range(2):
                        nc.tensor.matmul(
                            pso[:, e2 * 512:(e2 + 1) * 512],
                            lhsT=otb[mm][:, t16 * 128:(t16 + 1) * 128],
                            rhs=wo_sb[:, mm, e2 * 512:(e2 + 1) * 512],
                            start=(mm == 0), stop=(mm == 1))
                osb = outp.tile([128, 1024], FP32, tag="osb")
                if t16 % 2 == 0:
                    nc.vector.tensor_copy(out=osb, in_=pso)
                else:
                    nc.scalar.activation(out=osb, in_=pso,
                                         func=mybir.ActivationFunctionType.Copy)
                nc.sync.dma_start(
                    out=out_e.ap()[t16 * 128:(t16 + 1) * 128, :],
                    in_=osb)

            def attn_stream(m):
                for qb in range(4):
                    otps = [psu1.tile([128, 512], FP32, tag="u1",
                                      name=f"otp{m}{h_}") for h_ in range(2)]
                    nkb = 4 * qb + 4
                    exq = []
                    for kb in range(nkb):
                        st = psu.tile([128, 1024], FP32, tag="u", name="st")
                        for h in range(2):
                            pa = slice(64 * h, 64 * h + 64)
                            nc.tensor.matmul(
                                st[:, h * 512:(h + 1) * 512],
                                lhsT=kt[m][pa, kb * 128:(kb + 1) * 128],
                                rhs=qt[m][pa, qb * 512:(qb + 1) * 512],
                                start=True, stop=True)
                        ex = expp.tile([128, 1024], BF16, tag="exp")
                        nc.scalar.activation(
                            out=ex, in_=st,
                            func=mybir.ActivationFunctionType.Exp)
                        d = kb - 4 * qb
                        if d >= 0:  # diagonal block: causal 0/1 mask
                            nc.vector.tensor_tensor(
                                out=ex, in0=ex, in1=masks_sb[:, d, :],
                                op=mybir.AluOpType.mult)
                        exq.append((ex, kb))
                        if len(exq) > 3:
                            exp_, kb_ = exq.pop(0)
                            for h in range(2):
                                nc.tensor.matmul(
                                    otps[h][0:S + 1, :],
                                    lhsT=vhat[:, kb_, 2 * m + h, :],
                                    rhs=exp_[:, h * 512:(h + 1) * 512],
                                    start=(kb_ == 0), stop=False)
                        yield None
                    while exq:
                        exp_, kb_ = exq.pop(0)
                        for h in range(2):
                            nc.tensor.matmul(
                                otps[h][0:S + 1, :],
                                lhsT=vhat[:, kb_, 2 * m + h, :],
                                rhs=exp_[:, h * 512:(h + 1) * 512],
                                start=(kb_ == 0), stop=(kb_ == nkb - 1))
                    # evict O^T + sums rows, then finish normalize on-chip
                    otsb = otsbp.tile([128, 512], FP32, tag="otsb")
                    rcs = []
                    for h in range(2):
                        rc = rcpp.tile([1, 512], BF16, tag="rc", name=f"rc{h}")
                        nc.scalar.activation(out=rc, in_=otps[h][S:S + 1, :],
                                             func=mybir.ActivationFunctionType.Copy)
                        nc.scalar.activation(out=otsb[64 * h:64 * h + 64, :],
                                             in_=otps[h][0:S, :],
                                             func=mybir.ActivationFunctionType.Copy)
                        rcs.append(rc)
                    finish_norm((m, qb, otsb, rcs))
                    yield qb

            g0, g1 = attn_stream(0), attn_stream(1)
            next(g0)
            next(g0)
            done0 = done1 = False
            while not (done0 and done1):
                if not done0:
                    try:
                        next(g0)
                    except StopIteration:
                        done0 = True
                if not done1:
                    try:
                        next(g1)
                    except StopIteration:
                        done1 = True
            for t16 in range(16):
                emit_wo(t16)
    return nc


